# revision 2
# baseline (speedup 1.0000x reference)
"""TRN2 Bass kernel for nn_Attention_11407433138456.

Multi-head self-attention, B=4 Q=K=2048 D=1024 H=16 DH=64, fp32 inputs.

Sharding (8 cores): data-parallel over B (4 batches x 2 cores), tensor-
parallel over heads (2 groups of 8 heads). Core c handles batch c//2,
head group c%2. Each core computes its 8 heads' attention and a partial
output projection; the host sums the two partials per batch (+ bias).

Per-core dataflow:
  Qt/Kt [128,2048] f16 per head pair (rows 0:64 even head, 64:128 odd),
  via matmul(lhsT=W chunk, rhs=xT chunk) in f32r. Per (pair, key tile):
  logits^T [128k,1024q] in PSUM -> ACT exp -> f16 E tiles -> DVE mask
  multiply. PV runs output-stationary: matmul(out[128q, 65],
  lhsT=E[:,128q slice], rhs=V'[128k,65]) accumulated over 16 key tiles;
  V' column 64 is ones so the softmax denominator lands in the
  accumulator. DVE reciprocal + per-partition tensor_scalar normalize
  -> O [q,dh] f16 -> PE identity-transpose -> O^T [dh,q] -> output
  projection (f16) -> partial out [2048,1024] f32 DMA'd from PSUM.
"""

import os
from contextlib import ExitStack

import numpy as np

import concourse.bass as bass
import concourse.mybir as mybir
import concourse.tile as tile
from concourse import bacc
from concourse.bass_utils import run_bass_kernel_spmd

dt = mybir.dt
AF = mybir.ActivationFunctionType

B, Q, KS, D, H, DH = 4, 2048, 2048, 1024, 16, 64
DG = 512  # hidden slice per core (8 heads)
NPAIR = 4  # head pairs per core
NKT = KS // 128  # 16 key tiles
QW = 1024  # q block width for attention
NQT = Q // QW  # 2 q blocks
VW = 65  # V' per-head stride (64 dh + ones col)

_CACHE = {}


def _build(repeat=1, variant='full'):
    nc = bacc.Bacc("TRN2", target_bir_lowering=False, debug=False, num_devices=8)

    xT_d = nc.dram_tensor("xT", [D, Q], dt.float32r, kind="ExternalInput").ap()
    wq_d = nc.dram_tensor("wq", [D, DG], dt.float32r, kind="ExternalInput").ap()
    wk_d = nc.dram_tensor("wk", [D, DG], dt.float32r, kind="ExternalInput").ap()
    wv_d = nc.dram_tensor("wv", [D, DG], dt.float32r, kind="ExternalInput").ap()
    wo_d = nc.dram_tensor("wo", [DG, D], dt.float16, kind="ExternalInput").ap()
    mT_d = nc.dram_tensor("maskT", [KS, Q], dt.float16, kind="ExternalInput").ap()
    id_d = nc.dram_tensor("ident", [128, 128], dt.float16, kind="ExternalInput").ap()
    wq16_d = nc.dram_tensor("wq16", [D, DG], dt.float16, kind="ExternalInput").ap()
    xT16_d = nc.dram_tensor("xT16", [D, Q], dt.float16, kind="ExternalInput").ap()
    wk16_d = nc.dram_tensor("wk16", [D, DG], dt.float16, kind="ExternalInput").ap()
    out_d = nc.dram_tensor("out", [Q, D], dt.float32, kind="ExternalOutput").ap()

    with tile.TileContext(nc) as tc, ExitStack() as ctx:
        # ---- persistent pools ----
        qk_pool = ctx.enter_context(tc.tile_pool(name="qk", bufs=1))
        vv_pool = ctx.enter_context(tc.tile_pool(name="vv", bufs=1))
        ot_pool = ctx.enter_context(tc.tile_pool(name="ot", bufs=1))
        wo_pool = ctx.enter_context(tc.tile_pool(name="wop", bufs=1))
        psL = ctx.enter_context(tc.tile_pool(name="psL", bufs=2, space="PSUM"))
        psPV = ctx.enter_context(tc.tile_pool(name="psPV", bufs=1, space="PSUM"))
        psT = ctx.enter_context(tc.tile_pool(name="psT", bufs=1, space="PSUM"))
        psF = ctx.enter_context(tc.tile_pool(name="psF", bufs=2, space="PSUM"))

        # Qt/Kt: per pair, [128 dh, 2048 q] f16 (rows 0:64 even head,
        # 64:128 odd head)
        qt_sb = [qk_pool.tile([128, Q], dt.float16, name=f"qt{p}", tag=f"qt{p}") for p in range(NPAIR)]
        kt_sb = [qk_pool.tile([128, Q], dt.float16, name=f"kt{p}", tag=f"kt{p}") for p in range(NPAIR)]
        # V' per key tile: [128 keys, 8*66] f16, ones at col 64 of each head
        vv_sb = [vv_pool.tile([128, 8 * VW], dt.float16, name=f"vv{k}", tag=f"vv{k}") for k in range(NKT)]
        # O^T per pair: [128 dh, 2048 q] f16
        ot_sb = [ot_pool.tile([128, Q], dt.float16, name=f"ot{p}", tag=f"ot{p}") for p in range(NPAIR)]
        wo_sb = [wo_pool.tile([128, D], dt.float16, name=f"wo{c}", tag=f"wo{c}") for c in range(4)]
        id_sb = wo_pool.tile([128, 128], dt.float16, name="id", tag="id")

        for rep in range(repeat):
            # ---- phases A+B: projections, xT loaded in two q-halves ----
            # xa slots rotate: half 0 covers q/key cols 0:1024, half 1 the
            # rest. Q/K projections of pairs 1-3 for half 1 are deferred and
            # interleaved into the early attention head-phases.
            defer_ctx = ExitStack()
            xr_pool = defer_ctx.enter_context(tc.tile_pool(name=f"xr{rep}", bufs=1))
            wpf_pool = defer_ctx.enter_context(tc.tile_pool(name=f"wpf{rep}", bufs=2))
            proj_ctx = ExitStack()
            xa_pool = proj_ctx.enter_context(tc.tile_pool(name=f"xa{rep}", bufs=12))
            wpr_pool = proj_ctx.enter_context(tc.tile_pool(name=f"wpr{rep}", bufs=2))
            deferred = []

            def proj_unit_dma(p, w_d, to_f16=False):
                if to_f16:
                    wp = wpf_pool.tile([128, 1024], dt.float16, name="wpf", tag="wpf")
                else:
                    wp = wpr_pool.tile([128, 1024], dt.float32r, name="wp", tag="wp")
                nc.sync.dma_start(
                    out=wp[:].rearrange("p (c m) -> p c m", m=128),
                    in_=w_d[:, p * 128 : (p + 1) * 128].rearrange("(c p) m -> p c m", p=128),
                )
                return wp

            def proj_unit_mm(wp, dst, q0, xa, on_act, nts=(0, 1)):
                for nt in nts:
                    ps = psF.tile([128, 512], dt.float32, name="psA", tag="F")
                    for c in range(8):
                        nc.tensor.matmul(
                            ps[:],
                            lhsT=wp[:, c * 128 : (c + 1) * 128],
                            rhs=xa[c][:, nt * 512 : (nt + 1) * 512],
                            start=(c == 0),
                            stop=(c == 7),
                        )
                    d = dst[:, q0 + nt * 512 : q0 + (nt + 1) * 512]
                    if on_act:
                        nc.scalar.activation(d, ps[:], AF.Copy)
                    else:
                        nc.vector.tensor_copy(d, ps[:])

            with tc.tile_pool(name=f"wvp{rep}", bufs=1) as wv_pool:
                wv_sb = []
                for half in range(2):
                    q0 = half * 1024
                    xa = []
                    for c in range(8):
                        if half == 0:
                            w = wv_pool.tile(
                                [128, DG], dt.float32r, name=f"wv{c}", tag=f"wv{c}"
                            )
                            nc.sync.dma_start(
                                out=w[:], in_=wv_d[c * 128 : (c + 1) * 128, :]
                            )
                            wv_sb.append(w)
                        xt = xa_pool.tile([128, 1024], dt.float32r, name="xa", tag="xa")
                        nc.sync.dma_start(
                            out=xt[:], in_=xT_d[c * 128 : (c + 1) * 128, q0 : q0 + 1024]
                        )
                        xa.append(xt)
                    if half == 0 and rep == 0:
                        for c in range(4):
                            nc.sync.dma_start(
                                out=wo_sb[c][:], in_=wo_d[c * 128 : (c + 1) * 128, :]
                            )
                        nc.sync.dma_start(out=id_sb[:], in_=id_d[:, :])
                    # V projection for this half's key tiles
                    for k in ([] if variant == "qk_only" else range(8 * half, 8 * half + 8)):
                        kc = k * 128 - q0
                        ps = psF.tile([128, 512], dt.float32, name="psB", tag="F")
                        for c in range(8):
                            nc.tensor.matmul(
                                ps[:],
                                lhsT=xa[c][:, kc : kc + 128],
                                rhs=wv_sb[c][:],
                                start=(c == 0),
                                stop=(c == 7),
                            )
                        vvv = vv_sb[k][:].rearrange("p (h c) -> p h c", c=VW)
                        nc.vector.memset(vvv[:, :, 64:65], 1.0)
                        nc.vector.tensor_copy(
                            vvv[:, :, 0:64],
                            ps[:].rearrange("p (h c) -> p h c", c=64),
                        )
                    # Q/K projections for this half's q columns; pairs 1-3
                    # are deferred into the attention head-phases
                    for p in ([] if variant == "v_only" else range(NPAIR)):
                        for w_d, w16_d, dst in (
                            (wq_d, wq16_d, qt_sb[p]),
                            (wk_d, wk16_d, kt_sb[p]),
                        ):
                            if p > 0 and variant == "full":
                                if half == 0:
                                    # allocating touch-write: the real fills
                                    # are deferred into the attention phases
                                    nc.vector.memset(dst[:, 0:1], 0.0)
                                deferred.append((p, w16_d, dst, q0))
                                continue
                            wp = proj_unit_dma(p, w_d)
                            proj_unit_mm(wp, dst, q0, xa, on_act=True)
            proj_ctx.close()

            # ---- phase C: attention + output projection ----
            # Head-phases: one head at a time, 16 QK+exp slots each. PV
            # half-passes of the previous head, out-proj chunks of the
            # previous q block, and the deferred half-1 Q/K projections are
            # interleaved into the PE gaps left by the ACT-paced exp stream.
            # E tiles double-buffer by head parity.
            if variant in ("proj_only", "qk_only", "v_only"):
                defer_ctx.close()
                continue
            # deadline order: K before Q, pair-major; half0 before half1 of
            # the same pair (kt needs both halves by phase 2p, qt half1 only
            # by qt1). deferred was appended (half, pair, w)-major; re-sort:
            if deferred:
                def _dl(u):
                    p_, w_d_, dst_, q0_ = u
                    is_q = w_d_ is wq16_d
                    h1 = bool(q0_)
                    # kt (both halves) and qt-half0 due at phase 2p of qt0;
                    # qt-half1 due at phase 2p of qt1
                    return (2 * p_ + (16 if (is_q and h1) else 0), is_q, h1)
                deferred.sort(key=_dl)
            subq_all = [(i, nt) for i in range(len(deferred)) for nt in range(2)]
            defer_wp = {}
            xr_sb = {}
            sub_emitted = 0

            def defer_xr_load():
                """One [128, 8x1024] f16 x-slab per half, loaded once."""
                for q0_ in (0, 1024):
                    xr = xr_pool.tile([128, 8192], dt.float16, name="xr", tag=f"xr{q0_}")
                    nc.sync.dma_start(
                        out=xr[:].rearrange("p (c m) -> p c m", m=1024),
                        in_=xT16_d[:, q0_ : q0_ + 1024].rearrange(
                            "(c p) m -> p c m", p=128
                        ),
                    )
                    xr_sb[q0_] = xr

            def defer_emit():
                nonlocal sub_emitted
                i_, nt_ = subq_all[sub_emitted]
                p_, w16_d_, dst_, q0_ = deferred[i_]
                if i_ not in defer_wp:
                    defer_wp[i_] = proj_unit_dma(p_, w16_d_, to_f16=True)
                wp = defer_wp[i_]
                xr = xr_sb[q0_]
                ps = psF.tile([128, 512], dt.float32, name="psA", tag="F")
                for c in range(8):
                    nc.tensor.matmul(
                        ps[:],
                        lhsT=wp[:, c * 128 : (c + 1) * 128],
                        rhs=xr[:, c * 1024 + nt_ * 512 : c * 1024 + (nt_ + 1) * 512],
                        start=(c == 0),
                        stop=(c == 7),
                    )
                nc.vector.tensor_copy(
                    dst_[:, q0_ + nt_ * 512 : q0_ + (nt_ + 1) * 512], ps[:]
                )
                if nt_ == 1:
                    defer_wp.pop(i_)
                    if i_ + 1 < len(deferred):
                        defer_wp[i_ + 1] = proj_unit_dma(
                            deferred[i_ + 1][0], deferred[i_ + 1][1], to_f16=True
                        )
                sub_emitted += 1

            with tc.tile_pool(name=f"mask{rep}", bufs=16) as mask_pool, tc.tile_pool(
                name=f"et{rep}", bufs=1
            ) as et_pool, tc.tile_pool(name=f"ep{rep}", bufs=2) as ep_pool, tc.tile_pool(
                name=f"osb{rep}", bufs=1
            ) as osb_pool:
                pv_po = {}

                def emit_pv_halfpass(state, hp):
                    """Two contiguous 16-matmul accumulation chains; on the
                    closing half also normalize (+ transpose on odd heads)."""
                    sqt, hh, eset = state
                    p, ho = hh // 2, hh % 2
                    ps4, half = hp // 2, hp % 2
                    if half == 0:
                        pv_po["po"] = psPV.tile([128, 512], dt.float32, name="po", tag="PV")
                    po = pv_po["po"]
                    for sub in (0, 1) if half == 0 else (2, 3):
                        qc = ps4 * 4 + sub
                        for k in range(NKT):
                            nc.tensor.matmul(
                                po[:, sub * 128 : sub * 128 + 65],
                                lhsT=eset[k][:, qc * 128 : (qc + 1) * 128],
                                rhs=vv_sb[k][:, hh * VW : hh * VW + 65],
                                start=(k == 0),
                                stop=(k == NKT - 1),
                            )
                    if half == 0:
                        return
                    rec = ep_pool.tile([128, 4], dt.float32, name="rec", tag="rec")
                    nc.vector.reciprocal(
                        rec[:], po[:].rearrange("p (s c) -> p s c", c=128)[:, :, 64]
                    )
                    for sub in range(4):
                        qc = ps4 * 4 + sub
                        nc.vector.tensor_scalar_mul(
                            osb_t[qc][:, ho * 64 : (ho + 1) * 64],
                            po[:, sub * 128 : sub * 128 + 64],
                            rec[:, sub : sub + 1],
                        )
                    if ho == 1:
                        for sub in range(4):
                            qc = ps4 * 4 + sub
                            pt = psT.tile([128, 128], dt.float16, name="pt", tag="T")
                            nc.tensor.transpose(pt[:], osb_t[qc][:], id_sb[:])
                            nc.vector.tensor_copy(
                                ot_sb[p][:, sqt * QW + qc * 128 : sqt * QW + (qc + 1) * 128],
                                pt[:],
                            )
                        if hh == 7:
                            for sub in range(4):
                                op_queue.append((sqt, ps4 * 4 + sub))

                def emit_outproj(sqt, qc):
                    q0 = sqt * QW + qc * 128
                    for ncol in range(2):
                        pf = psF.tile([128, 512], dt.float32, name="psF", tag="F")
                        for p in range(NPAIR):
                            nc.tensor.matmul(
                                pf[:],
                                lhsT=ot_sb[p][:, q0 : q0 + 128],
                                rhs=wo_sb[p][:, ncol * 512 : (ncol + 1) * 512],
                                start=(p == 0),
                                stop=(p == NPAIR - 1),
                            )
                        fsb = wpf_pool.tile([128, 512], dt.float32, name="fsb", tag="wpf")
                        nc.vector.tensor_copy(fsb[:], pf[:])
                        nc.sync.dma_start(
                            out=out_d[q0 : q0 + 128, ncol * 512 : (ncol + 1) * 512],
                            in_=fsb[:],
                        )

                prev = None
                op_queue = []
                osb_t = [
                    osb_pool.tile([128, 128], dt.float16, name="osb", tag=f"o{i}")
                    for i in range(QW // 128)
                ]
                for qt in range(NQT):
                    mtiles = []
                    for k in range(NKT):
                        mt = mask_pool.tile([128, QW], dt.float16, name="mt", tag="mt")
                        nc.sync.dma_start(
                            out=mt[:], in_=mT_d[k * 128 : (k + 1) * 128, qt * QW : (qt + 1) * QW]
                        )
                        mtiles.append(mt)
                        if qt == 0 and k == 3 and deferred:
                            defer_xr_load()
                    for hh in range(8):
                        p, ho = hh // 2, hh % 2
                        b0 = ho * 64
                        eset = {}
                        for k in range(NKT):
                            pl = psL.tile([128, QW], dt.float32, name="psL", tag="L")
                            for hf in range(QW // 512):
                                nc.tensor.matmul(
                                    pl[:, hf * 512 : (hf + 1) * 512],
                                    lhsT=kt_sb[p][b0 : b0 + 64, k * 128 : (k + 1) * 128],
                                    rhs=qt_sb[p][
                                        b0 : b0 + 64,
                                        qt * QW + hf * 512 : qt * QW + (hf + 1) * 512,
                                    ],
                                    start=True,
                                    stop=True,
                                )
                            e = et_pool.tile(
                                [128, QW], dt.float16, name="et", tag=f"e{k}_{hh % 2}"
                            )
                            nc.scalar.activation(
                                e[:], pl[:], AF.Copy if variant == "no_exp" else AF.Exp
                            )
                            if variant != "no_mask":
                                nc.gpsimd.tensor_mul(e[:], e[:], mtiles[k][:])
                            eset[k] = e
                            if k % 4 == 3 and prev is not None:
                                emit_pv_halfpass(prev, k // 4)
                            if k in (6, 14) and op_queue:
                                emit_outproj(*op_queue.pop(0))
                            eligible = (
                                k in (7, 9, 11, 13, 15)
                                if (qt == 0 and hh == 0)
                                else k in (1, 2, 5, 6, 9, 10)
                            )
                            if sub_emitted < len(subq_all) and eligible:
                                defer_emit()
                        prev = (qt, hh, eset)
                # tail: last head's PV, then remaining out-proj chunks
                for hp in range(4):
                    emit_pv_halfpass(prev, hp)
                    if hp >= 1:
                        for _ in range(2):
                            if op_queue:
                                emit_outproj(*op_queue.pop(0))
                while op_queue:
                    emit_outproj(*op_queue.pop(0))
            defer_ctx.close()

    nc.compile()
    return nc


def _get_nc():
    if "nc" not in _CACHE:
        _CACHE["nc"] = _build()
    return _CACHE["nc"]


def kernel(x, mask, Wq, Wk, Wv, Wo, bo):
    x = np.asarray(x, dtype=np.float32)
    mask_f16 = np.asarray(mask).astype(np.float16)
    Wq = np.asarray(Wq, dtype=np.float32)
    Wk = np.asarray(Wk, dtype=np.float32)
    Wv = np.asarray(Wv, dtype=np.float32)
    Wo = np.asarray(Wo, dtype=np.float32)
    bo = np.asarray(bo, dtype=np.float32)

    scale = np.float32(DH**-0.5)
    ident = np.eye(128, dtype=np.float16)
    nc = _get_nc()

    in_maps = []
    for c in range(8):
        b, g = c // 2, c % 2
        gs = slice(g * DG, (g + 1) * DG)
        in_maps.append(
            {
                "xT": np.ascontiguousarray(x[b].T),
                "wq": np.ascontiguousarray(Wq[:, gs]) * scale,
                "wk": np.ascontiguousarray(Wk[:, gs]),
                "wv": np.ascontiguousarray(Wv[:, gs]),
                "wo": np.ascontiguousarray(Wo[gs, :]).astype(np.float16),
                "maskT": np.ascontiguousarray(mask_f16[b].T),
                "ident": ident,
                "wq16": (np.ascontiguousarray(Wq[:, gs]) * scale).astype(np.float16),
                "wk16": np.ascontiguousarray(Wk[:, gs]).astype(np.float16),
                "xT16": np.ascontiguousarray(x[b].T).astype(np.float16),
            }
        )

    res = run_bass_kernel_spmd(nc, in_maps, list(range(8))).results

    out = np.empty((B, Q, D), dtype=np.float32)
    for b in range(B):
        out[b] = res[2 * b]["out"] + res[2 * b + 1]["out"]
    out += bo
    return out



# revision 12
# speedup vs baseline: 1.6421x; 1.6421x over previous
"""TRN2 Bass kernel for nn_Attention_11407433138456.

Multi-head self-attention, B=4 Q=K=2048 D=1024 H=16 DH=64, fp32 inputs.

Sharding (8 cores): data-parallel over B (4 batches x 2 cores), tensor-
parallel over heads (2 groups of 8 heads). Core c handles batch c//2,
head group c%2. Each core computes its 8 heads' attention and a partial
output projection; the host sums the two partials per batch (+ bias).

Per-core dataflow:
  Qt/Kt [128,2048] f16 per head pair (rows 0:64 even head, 64:128 odd),
  via matmul(lhsT=W chunk, rhs=xT chunk) in f32r. Per (pair, key tile):
  logits^T [128k,1024q] in PSUM -> ACT exp -> f16 E tiles -> DVE mask
  multiply. PV runs output-stationary: matmul(out[128q, 65],
  lhsT=E[:,128q slice], rhs=V'[128k,65]) accumulated over 16 key tiles;
  V' column 64 is ones so the softmax denominator lands in the
  accumulator. DVE reciprocal + per-partition tensor_scalar normalize
  -> O [q,dh] f16 -> PE identity-transpose -> O^T [dh,q] -> output
  projection (f16) -> partial out [2048,1024] f32 DMA'd from PSUM.
"""

import os
from contextlib import ExitStack

import numpy as np

import concourse.bass as bass
import concourse.mybir as mybir
import concourse.tile as tile
from concourse import bacc
from concourse.bass_utils import run_bass_kernel_spmd

dt = mybir.dt
AF = mybir.ActivationFunctionType

B, Q, KS, D, H, DH = 4, 2048, 2048, 1024, 16, 64
DG = 512  # hidden slice per core (8 heads)
NPAIR = 4  # head pairs per core
NKT = KS // 128  # 16 key tiles
QW = 1024  # q block width for attention
NQT = Q // QW  # 2 q blocks
VW = 65  # V' per-head stride (64 dh + ones col)

_CACHE = {}


def _build(repeat=1, variant='full'):
    nc = bacc.Bacc("TRN2", target_bir_lowering=False, debug=False, num_devices=8)

    xT_d = nc.dram_tensor("xT", [D, Q], dt.float32r, kind="ExternalInput").ap()
    wq_d = nc.dram_tensor("wq", [D, DG], dt.float32r, kind="ExternalInput").ap()
    wk_d = nc.dram_tensor("wk", [D, DG], dt.float32r, kind="ExternalInput").ap()
    wv_d = nc.dram_tensor("wv", [D, DG], dt.float32r, kind="ExternalInput").ap()
    wo_d = nc.dram_tensor("wo", [DG, D], dt.float16, kind="ExternalInput").ap()
    mT_d = nc.dram_tensor("maskT", [KS, Q], dt.float16, kind="ExternalInput").ap()
    wq16_d = nc.dram_tensor("wq16", [D, DG], dt.float16, kind="ExternalInput").ap()
    xT16_d = nc.dram_tensor("xT16", [D, Q], dt.float16, kind="ExternalInput").ap()
    wk16_d = nc.dram_tensor("wk16", [D, DG], dt.float16, kind="ExternalInput").ap()
    out_d = nc.dram_tensor("out", [Q, D], dt.float16, kind="ExternalOutput").ap()

    with tile.TileContext(nc) as tc, ExitStack() as ctx:
        # ---- persistent pools ----
        qk_pool = ctx.enter_context(tc.tile_pool(name="qk", bufs=1))
        vv_pool = ctx.enter_context(tc.tile_pool(name="vv", bufs=1))
        ot_pool = ctx.enter_context(tc.tile_pool(name="ot", bufs=1))
        wo_pool = ctx.enter_context(tc.tile_pool(name="wop", bufs=1))
        psL = ctx.enter_context(tc.tile_pool(name="psL", bufs=2, space="PSUM"))
        psPV = ctx.enter_context(tc.tile_pool(name="psPV", bufs=2, space="PSUM"))
        psF = ctx.enter_context(tc.tile_pool(name="psF", bufs=2, space="PSUM"))

        # Qt/Kt: per pair, [128 dh, 2048 q] f16 (rows 0:64 even head,
        # 64:128 odd head)
        qt_sb = [qk_pool.tile([128, Q], dt.float16, name=f"qt{p}", tag=f"qt{p}") for p in range(NPAIR)]
        kt_sb = [qk_pool.tile([128, Q], dt.float16, name=f"kt{p}", tag=f"kt{p}") for p in range(NPAIR)]
        # V' per key tile: [128 keys, 8*66] f16, ones at col 64 of each head
        vv_sb = [vv_pool.tile([128, 8 * VW], dt.float16, name=f"vv{k}", tag=f"vv{k}") for k in range(NKT)]
        # O^T per pair: [128 dh, 2048 q] f16
        ot_sb = [ot_pool.tile([128, Q], dt.float16, name=f"ot{p}", tag=f"ot{p}") for p in range(NPAIR)]
        wo_sb = [wo_pool.tile([128, D], dt.float16, name=f"wo{c}", tag=f"wo{c}") for c in range(4)]

        for rep in range(repeat):
            # ---- phases A+B: projections, xT loaded in two q-halves ----
            # xa slots rotate: half 0 covers q/key cols 0:1024, half 1 the
            # rest. Q/K projections of pairs 1-3 for half 1 are deferred and
            # interleaved into the early attention head-phases.
            defer_ctx = ExitStack()
            xr_pool = defer_ctx.enter_context(tc.tile_pool(name=f"xr{rep}", bufs=1))
            wpf_pool = defer_ctx.enter_context(tc.tile_pool(name=f"wpf{rep}", bufs=2))
            proj_ctx = ExitStack()
            xa_pool = proj_ctx.enter_context(tc.tile_pool(name=f"xa{rep}", bufs=12))
            wpr_pool = proj_ctx.enter_context(tc.tile_pool(name=f"wpr{rep}", bufs=2))
            deferred = []

            def proj_unit_dma(p, w_d, to_f16=False):
                if to_f16:
                    wp = wpf_pool.tile([128, 1024], dt.float16, name="wpf", tag="wpf")
                else:
                    wp = wpr_pool.tile([128, 1024], dt.float32r, name="wp", tag="wp")
                nc.sync.dma_start(
                    out=wp[:].rearrange("p (c m) -> p c m", m=128),
                    in_=w_d[:, p * 128 : (p + 1) * 128].rearrange("(c p) m -> p c m", p=128),
                )
                return wp

            def proj_unit_mm(wp, dst, q0, xa, on_act, nts=(0, 1)):
                for nt in nts:
                    ps = psF.tile([128, 512], dt.float32, name="psA", tag="F")
                    for c in range(8):
                        nc.tensor.matmul(
                            ps[:],
                            lhsT=wp[:, c * 128 : (c + 1) * 128],
                            rhs=xa[c][:, nt * 512 : (nt + 1) * 512],
                            start=(c == 0),
                            stop=(c == 7),
                        )
                    d = dst[:, q0 + nt * 512 : q0 + (nt + 1) * 512]
                    if on_act:
                        nc.scalar.activation(d, ps[:], AF.Copy)
                    else:
                        nc.vector.tensor_copy(d, ps[:])

            with tc.tile_pool(name=f"wvp{rep}", bufs=1) as wv_pool:
                wv_sb = []
                for half in range(2):
                    q0 = half * 1024
                    xa = []
                    for c in range(8):
                        if half == 0:
                            w = wv_pool.tile(
                                [128, DG], dt.float32r, name=f"wv{c}", tag=f"wv{c}"
                            )
                            nc.sync.dma_start(
                                out=w[:], in_=wv_d[c * 128 : (c + 1) * 128, :]
                            )
                            wv_sb.append(w)
                        xt = xa_pool.tile([128, 1024], dt.float32r, name="xa", tag="xa")
                        nc.sync.dma_start(
                            out=xt[:], in_=xT_d[c * 128 : (c + 1) * 128, q0 : q0 + 1024]
                        )
                        xa.append(xt)
                    if half == 0 and rep == 0:
                        for c in range(4):
                            nc.sync.dma_start(
                                out=wo_sb[c][:], in_=wo_d[c * 128 : (c + 1) * 128, :]
                            )
                    # V projection for this half's key tiles
                    for k in ([] if variant == "qk_only" else range(8 * half, 8 * half + 8)):
                        kc = k * 128 - q0
                        ps = psF.tile([128, 512], dt.float32, name="psB", tag="F")
                        for c in range(8):
                            nc.tensor.matmul(
                                ps[:],
                                lhsT=xa[c][:, kc : kc + 128],
                                rhs=wv_sb[c][:],
                                start=(c == 0),
                                stop=(c == 7),
                            )
                        vvv = vv_sb[k][:].rearrange("p (h c) -> p h c", c=VW)
                        nc.vector.memset(vvv[:, :, 64:65], 1.0)
                        nc.vector.tensor_copy(
                            vvv[:, :, 0:64],
                            ps[:].rearrange("p (h c) -> p h c", c=64),
                        )
                    # Q/K projections for this half's q columns; pairs 1-3
                    # are deferred into the attention head-phases
                    for p in ([] if variant == "v_only" else range(NPAIR)):
                        for w_d, w16_d, dst in (
                            (wq_d, wq16_d, qt_sb[p]),
                            (wk_d, wk16_d, kt_sb[p]),
                        ):
                            if p > 0 and variant == "full":
                                if half == 0:
                                    # allocating touch-write: the real fills
                                    # are deferred into the attention phases
                                    nc.vector.memset(dst[:, 0:1], 0.0)
                                deferred.append((p, w16_d, dst, q0))
                                continue
                            wp = proj_unit_dma(p, w_d)
                            proj_unit_mm(wp, dst, q0, xa, on_act=True)
            proj_ctx.close()

            # ---- phase C: attention + output projection ----
            # Head-phases: one head at a time, 16 QK+exp slots each. PV
            # half-passes of the previous head, out-proj chunks of the
            # previous q block, and the deferred half-1 Q/K projections are
            # interleaved into the PE gaps left by the ACT-paced exp stream.
            # E tiles double-buffer by head parity.
            if variant in ("proj_only", "qk_only", "v_only"):
                defer_ctx.close()
                continue
            # deadline order: K before Q, pair-major; half0 before half1 of
            # the same pair (kt needs both halves by phase 2p, qt half1 only
            # by qt1). deferred was appended (half, pair, w)-major; re-sort:
            if deferred:
                def _dl(u):
                    p_, w_d_, dst_, q0_ = u
                    is_q = w_d_ is wq16_d
                    h1 = bool(q0_)
                    # kt (both halves) and qt-half0 due at phase 2p of qt0;
                    # qt-half1 due at phase 2p of qt1
                    return (2 * p_ + (16 if (is_q and h1) else 0), is_q, h1)
                deferred.sort(key=_dl)
            subq_all = [(i, nt) for i in range(len(deferred)) for nt in range(2)]
            defer_wp = {}
            xr_sb = {}
            sub_emitted = 0

            def defer_xr_load():
                """One [128, 8x1024] f16 x-slab per half, loaded once."""
                for q0_ in (0, 1024):
                    xr = xr_pool.tile([128, 8192], dt.float16, name="xr", tag=f"xr{q0_}")
                    nc.sync.dma_start(
                        out=xr[:].rearrange("p (c m) -> p c m", m=1024),
                        in_=xT16_d[:, q0_ : q0_ + 1024].rearrange(
                            "(c p) m -> p c m", p=128
                        ),
                    )
                    xr_sb[q0_] = xr

            def defer_emit():
                nonlocal sub_emitted
                i_, nt_ = subq_all[sub_emitted]
                p_, w16_d_, dst_, q0_ = deferred[i_]
                if i_ not in defer_wp:
                    defer_wp[i_] = proj_unit_dma(p_, w16_d_, to_f16=True)
                wp = defer_wp[i_]
                xr = xr_sb[q0_]
                ps = psF.tile([128, 512], dt.float32, name="psA", tag="F")
                for c in range(8):
                    nc.tensor.matmul(
                        ps[:],
                        lhsT=wp[:, c * 128 : (c + 1) * 128],
                        rhs=xr[:, c * 1024 + nt_ * 512 : c * 1024 + (nt_ + 1) * 512],
                        start=(c == 0),
                        stop=(c == 7),
                    )
                nc.vector.tensor_copy(
                    dst_[:, q0_ + nt_ * 512 : q0_ + (nt_ + 1) * 512], ps[:]
                )
                if nt_ == 1:
                    defer_wp.pop(i_)
                    if i_ + 1 < len(deferred):
                        defer_wp[i_ + 1] = proj_unit_dma(
                            deferred[i_ + 1][0], deferred[i_ + 1][1], to_f16=True
                        )
                sub_emitted += 1

            with tc.tile_pool(name=f"mask{rep}", bufs=16) as mask_pool, tc.tile_pool(
                name=f"et{rep}", bufs=1
            ) as et_pool, tc.tile_pool(name=f"ep{rep}", bufs=2) as ep_pool, tc.tile_pool(
                name=f"osb{rep}", bufs=1
            ) as osb_pool:
                pv_po = {}

                def emit_pv_halfpass(state, hp):
                    """Two contiguous 16-matmul accumulation chains; on the
                    closing half also normalize (+ transpose on odd heads)."""
                    sqt, hh, eset = state
                    p, ho = hh // 2, hh % 2
                    ps4, half = hp // 2, hp % 2
                    if half == 0:
                        pv_po["po"] = psPV.tile([128, 512], dt.float32, name="po", tag="PV")
                    po = pv_po["po"]
                    for sub in (0, 1) if half == 0 else (2, 3):
                        qc = ps4 * 4 + sub
                        for k in range(NKT):
                            nc.tensor.matmul(
                                po[:, sub * 128 : sub * 128 + 65],
                                lhsT=eset[k][:, qc * 128 : (qc + 1) * 128],
                                rhs=vv_sb[k][:, hh * VW : hh * VW + 65],
                                start=(k == 0),
                                stop=(k == NKT - 1),
                            )
                    if half == 0:
                        return
                    rec = ep_pool.tile([128, 4], dt.float32, name="rec", tag="rec")
                    nc.vector.reciprocal(
                        rec[:], po[:].rearrange("p (s c) -> p s c", c=128)[:, :, 64]
                    )
                    for sub in range(4):
                        qc = ps4 * 4 + sub
                        nc.vector.tensor_scalar_mul(
                            osb_t[qc][:, ho * 64 : (ho + 1) * 64],
                            po[:, sub * 128 : sub * 128 + 64],
                            rec[:, sub : sub + 1],
                        )
                    if ho == 1:
                        for sub in range(4):
                            qc = ps4 * 4 + sub
                            nc.sync.dma_start_transpose(
                                out=ot_sb[p][:, sqt * QW + qc * 128 : sqt * QW + (qc + 1) * 128],
                                in_=osb_t[qc][:],
                            )
                        if hh == 7:
                            for sub in range(4):
                                op_queue.append((sqt, ps4 * 4 + sub))

                def emit_outproj(sqt, qc):
                    q0 = sqt * QW + qc * 128
                    for ncol in range(2):
                        pf = psF.tile([128, 512], dt.float32, name="psF", tag="F")
                        for p in range(NPAIR):
                            nc.tensor.matmul(
                                pf[:],
                                lhsT=ot_sb[p][:, q0 : q0 + 128],
                                rhs=wo_sb[p][:, ncol * 512 : (ncol + 1) * 512],
                                start=(p == 0),
                                stop=(p == NPAIR - 1),
                            )
                        fsb = wpf_pool.tile([128, 512], dt.float16, name="fsb", tag="wpf")
                        nc.vector.tensor_copy(fsb[:], pf[:])
                        nc.sync.dma_start(
                            out=out_d[q0 : q0 + 128, ncol * 512 : (ncol + 1) * 512],
                            in_=fsb[:],
                        )

                prev = None
                op_queue = []
                osb_t = [
                    osb_pool.tile([128, 128], dt.float16, name="osb", tag=f"o{i}")
                    for i in range(QW // 128)
                ]
                for qt in range(NQT):
                    mtiles = []
                    for k in range(NKT):
                        mt = mask_pool.tile([128, QW], dt.float16, name="mt", tag="mt")
                        nc.sync.dma_start(
                            out=mt[:], in_=mT_d[k * 128 : (k + 1) * 128, qt * QW : (qt + 1) * QW]
                        )
                        mtiles.append(mt)
                        if qt == 0 and k == 3 and deferred:
                            defer_xr_load()
                    for hh in range(8):
                        p, ho = hh // 2, hh % 2
                        b0 = ho * 64
                        eset = {}
                        for k in range(NKT):
                            pl = psL.tile([128, QW], dt.float32, name="psL", tag="L")
                            for hf in range(QW // 512):
                                nc.tensor.matmul(
                                    pl[:, hf * 512 : (hf + 1) * 512],
                                    lhsT=kt_sb[p][b0 : b0 + 64, k * 128 : (k + 1) * 128],
                                    rhs=qt_sb[p][
                                        b0 : b0 + 64,
                                        qt * QW + hf * 512 : qt * QW + (hf + 1) * 512,
                                    ],
                                    start=True,
                                    stop=True,
                                )
                            e = et_pool.tile(
                                [128, QW], dt.float16, name="et", tag=f"e{k}_{hh % 2}"
                            )
                            nc.scalar.activation(
                                e[:], pl[:], AF.Copy if variant == "no_exp" else AF.Exp
                            )
                            if variant != "no_mask":
                                nc.vector.tensor_mul(e[:], e[:], mtiles[k][:])
                            eset[k] = e
                            if k % 4 == 3 and prev is not None:
                                emit_pv_halfpass(prev, k // 4)
                            if k in (6, 14) and op_queue:
                                emit_outproj(*op_queue.pop(0))
                            eligible = (
                                k in (7, 9, 11, 13, 15)
                                if (qt == 0 and hh == 0)
                                else k in (1, 2, 5, 6, 9, 10)
                            )
                            if sub_emitted < len(subq_all) and eligible:
                                defer_emit()
                        prev = (qt, hh, eset)
                # tail: last head's PV, then remaining out-proj chunks
                for hp in range(4):
                    emit_pv_halfpass(prev, hp)
                    if hp >= 1:
                        for _ in range(2):
                            if op_queue:
                                emit_outproj(*op_queue.pop(0))
                while op_queue:
                    emit_outproj(*op_queue.pop(0))
            defer_ctx.close()

    nc.compile()
    return nc


def _get_nc():
    if "nc" not in _CACHE:
        _CACHE["nc"] = _build()
    return _CACHE["nc"]


def kernel(x, mask, Wq, Wk, Wv, Wo, bo):
    x = np.asarray(x, dtype=np.float32)
    mask_f16 = np.asarray(mask).astype(np.float16)
    Wq = np.asarray(Wq, dtype=np.float32)
    Wk = np.asarray(Wk, dtype=np.float32)
    Wv = np.asarray(Wv, dtype=np.float32)
    Wo = np.asarray(Wo, dtype=np.float32)
    bo = np.asarray(bo, dtype=np.float32)

    scale = np.float32(DH**-0.5)
    nc = _get_nc()

    in_maps = []
    for c in range(8):
        b, g = c // 2, c % 2
        gs = slice(g * DG, (g + 1) * DG)
        in_maps.append(
            {
                "xT": np.ascontiguousarray(x[b].T),
                "wq": np.ascontiguousarray(Wq[:, gs]) * scale,
                "wk": np.ascontiguousarray(Wk[:, gs]),
                "wv": np.ascontiguousarray(Wv[:, gs]),
                "wo": np.ascontiguousarray(Wo[gs, :]).astype(np.float16),
                "maskT": np.ascontiguousarray(mask_f16[b].T),
                "wq16": (np.ascontiguousarray(Wq[:, gs]) * scale).astype(np.float16),
                "wk16": np.ascontiguousarray(Wk[:, gs]).astype(np.float16),
                "xT16": np.ascontiguousarray(x[b].T).astype(np.float16),
            }
        )

    res = run_bass_kernel_spmd(nc, in_maps, list(range(8))).results

    out = np.empty((B, Q, D), dtype=np.float32)
    for b in range(B):
        out[b] = res[2 * b]["out"].astype(np.float32) + res[2 * b + 1]["out"].astype(
            np.float32
        )
    out += bo
    return out



# revision 19
# speedup vs baseline: 1.6453x; 1.0020x over previous
"""TRN2 Bass kernel for nn_Attention_11407433138456.

Multi-head self-attention, B=4 Q=K=2048 D=1024 H=16 DH=64, fp32 inputs.

Sharding (8 cores): data-parallel over B (4 batches x 2 cores), tensor-
parallel over heads (2 groups of 8 heads). Core c handles batch c//2,
head group c%2. Each core computes its 8 heads' attention and a partial
output projection; the host sums the two partials per batch (+ bias).

Per-core dataflow:
  Qt/Kt [128,2048] f16 per head pair (rows 0:64 even head, 64:128 odd),
  via matmul(lhsT=W chunk, rhs=xT chunk) in f32r. Per (pair, key tile):
  logits^T [128k,1024q] in PSUM -> ACT exp -> f16 E tiles -> DVE mask
  multiply. PV runs output-stationary: matmul(out[128q, 65],
  lhsT=E[:,128q slice], rhs=V'[128k,65]) accumulated over 16 key tiles;
  V' column 64 is ones so the softmax denominator lands in the
  accumulator. DVE reciprocal + per-partition tensor_scalar normalize
  -> O [q,dh] f16 -> PE identity-transpose -> O^T [dh,q] -> output
  projection (f16) -> partial out [2048,1024] f32 DMA'd from PSUM.
"""

import os
from contextlib import ExitStack

import numpy as np

import concourse.bass as bass
import concourse.mybir as mybir
import concourse.tile as tile
from concourse import bacc
from concourse.bass_utils import run_bass_kernel_spmd

dt = mybir.dt
AF = mybir.ActivationFunctionType

B, Q, KS, D, H, DH = 4, 2048, 2048, 1024, 16, 64
DG = 512  # hidden slice per core (8 heads)
NPAIR = 4  # head pairs per core
NKT = KS // 128  # 16 key tiles
QW = 1024  # q block width for attention
NQT = Q // QW  # 2 q blocks
VW = 65  # V' per-head stride (64 dh + ones col)

_CACHE = {}


def _build(repeat=1, variant='full'):
    nc = bacc.Bacc("TRN2", target_bir_lowering=False, debug=False, num_devices=8)

    xT_d = nc.dram_tensor("xT", [D, Q], dt.float32r, kind="ExternalInput").ap()
    wq_d = nc.dram_tensor("wq", [D, DG], dt.float32r, kind="ExternalInput").ap()
    wk_d = nc.dram_tensor("wk", [D, DG], dt.float32r, kind="ExternalInput").ap()
    wv_d = nc.dram_tensor("wv", [D, DG], dt.float32r, kind="ExternalInput").ap()
    wo_d = nc.dram_tensor("wo", [DG, D], dt.float16, kind="ExternalInput").ap()
    mT_d = nc.dram_tensor("maskT", [KS, Q], dt.float16, kind="ExternalInput").ap()
    wq16_d = nc.dram_tensor("wq16", [D, DG], dt.float16, kind="ExternalInput").ap()
    xT16_d = nc.dram_tensor("xT16", [D, Q], dt.float16, kind="ExternalInput").ap()
    wk16_d = nc.dram_tensor("wk16", [D, DG], dt.float16, kind="ExternalInput").ap()
    out_d = nc.dram_tensor("out", [Q, D], dt.float16, kind="ExternalOutput").ap()

    with tile.TileContext(nc) as tc, ExitStack() as ctx:
        # ---- persistent pools ----
        qk_pool = ctx.enter_context(tc.tile_pool(name="qk", bufs=1))
        vv_pool = ctx.enter_context(tc.tile_pool(name="vv", bufs=1))
        ot_pool = ctx.enter_context(tc.tile_pool(name="ot", bufs=1))
        wo_pool = ctx.enter_context(tc.tile_pool(name="wop", bufs=1))
        psL = ctx.enter_context(tc.tile_pool(name="psL", bufs=2, space="PSUM"))
        psPV = ctx.enter_context(tc.tile_pool(name="psPV", bufs=2, space="PSUM"))
        psF = ctx.enter_context(tc.tile_pool(name="psF", bufs=2, space="PSUM"))

        # Qt/Kt: per pair, [128 dh, 2048 q] f16 (rows 0:64 even head,
        # 64:128 odd head)
        qt_sb = [qk_pool.tile([128, Q], dt.float16, name=f"qt{p}", tag=f"qt{p}") for p in range(NPAIR)]
        kt_sb = [qk_pool.tile([128, Q], dt.float16, name=f"kt{p}", tag=f"kt{p}") for p in range(NPAIR)]
        # V' per key tile: [128 keys, 8*66] f16, ones at col 64 of each head
        vv_sb = [vv_pool.tile([128, 8 * VW], dt.float16, name=f"vv{k}", tag=f"vv{k}") for k in range(NKT)]
        # O^T per pair: [128 dh, 2048 q] f16
        ot_sb = [ot_pool.tile([128, Q], dt.float16, name=f"ot{p}", tag=f"ot{p}") for p in range(NPAIR)]
        wo_sb = [wo_pool.tile([128, D], dt.float16, name=f"wo{c}", tag=f"wo{c}") for c in range(4)]

        for rep in range(repeat):
            # ---- phases A+B: projections, xT loaded in two q-halves ----
            # xa slots rotate: half 0 covers q/key cols 0:1024, half 1 the
            # rest. Q/K projections of pairs 1-3 for half 1 are deferred and
            # interleaved into the early attention head-phases.
            defer_ctx = ExitStack()
            xr_pool = defer_ctx.enter_context(tc.tile_pool(name=f"xr{rep}", bufs=1))
            wpf_pool = defer_ctx.enter_context(tc.tile_pool(name=f"wpf{rep}", bufs=2))
            proj_ctx = ExitStack()
            xa_pool = proj_ctx.enter_context(tc.tile_pool(name=f"xa{rep}", bufs=12))
            wpr_pool = proj_ctx.enter_context(tc.tile_pool(name=f"wpr{rep}", bufs=2))
            deferred = []

            def proj_unit_dma(p, w_d, to_f16=False):
                if to_f16:
                    wp = wpf_pool.tile([128, 1024], dt.float16, name="wpf", tag="wpf")
                else:
                    wp = wpr_pool.tile([128, 1024], dt.float32r, name="wp", tag="wp")
                nc.sync.dma_start(
                    out=wp[:].rearrange("p (c m) -> p c m", m=128),
                    in_=w_d[:, p * 128 : (p + 1) * 128].rearrange("(c p) m -> p c m", p=128),
                )
                return wp

            def proj_unit_mm(wp, dst, q0, xa, on_act, nts=(0, 1)):
                for nt in nts:
                    ps = psF.tile([128, 512], dt.float32, name="psA", tag="F")
                    for c in range(8):
                        nc.tensor.matmul(
                            ps[:],
                            lhsT=wp[:, c * 128 : (c + 1) * 128],
                            rhs=xa[c][:, nt * 512 : (nt + 1) * 512],
                            start=(c == 0),
                            stop=(c == 7),
                        )
                    d = dst[:, q0 + nt * 512 : q0 + (nt + 1) * 512]
                    if on_act:
                        nc.scalar.activation(d, ps[:], AF.Copy)
                    else:
                        nc.vector.tensor_copy(d, ps[:])

            with tc.tile_pool(name=f"wvp{rep}", bufs=1) as wv_pool:
                wv_sb = []
                for half in range(2):
                    q0 = half * 1024
                    xa = []
                    for c in range(8):
                        if half == 0:
                            w = wv_pool.tile(
                                [128, DG], dt.float32r, name=f"wv{c}", tag=f"wv{c}"
                            )
                            nc.sync.dma_start(
                                out=w[:], in_=wv_d[c * 128 : (c + 1) * 128, :]
                            )
                            wv_sb.append(w)
                        xt = xa_pool.tile([128, 1024], dt.float32r, name="xa", tag="xa")
                        nc.sync.dma_start(
                            out=xt[:], in_=xT_d[c * 128 : (c + 1) * 128, q0 : q0 + 1024]
                        )
                        xa.append(xt)
                    if half == 0 and rep == 0:
                        for c in range(4):
                            nc.sync.dma_start(
                                out=wo_sb[c][:], in_=wo_d[c * 128 : (c + 1) * 128, :]
                            )
                    # V projection for this half's key tiles
                    for k in ([] if variant == "qk_only" else range(8 * half, 8 * half + 8)):
                        kc = k * 128 - q0
                        ps = psF.tile([128, 512], dt.float32, name="psB", tag="F")
                        for c in range(8):
                            nc.tensor.matmul(
                                ps[:],
                                lhsT=xa[c][:, kc : kc + 128],
                                rhs=wv_sb[c][:],
                                start=(c == 0),
                                stop=(c == 7),
                            )
                        vvv = vv_sb[k][:].rearrange("p (h c) -> p h c", c=VW)
                        nc.vector.memset(vvv[:, :, 64:65], 1.0)
                        nc.vector.tensor_copy(
                            vvv[:, :, 0:64],
                            ps[:].rearrange("p (h c) -> p h c", c=64),
                        )
                    # Q/K projections for this half's q columns; pairs 1-3
                    # are deferred into the attention head-phases
                    for p in ([] if variant == "v_only" else range(NPAIR)):
                        for w_d, w16_d, dst in (
                            (wq_d, wq16_d, qt_sb[p]),
                            (wk_d, wk16_d, kt_sb[p]),
                        ):
                            if p > 0 and variant == "full":
                                if half == 0:
                                    # allocating touch-write: the real fills
                                    # are deferred into the attention phases
                                    nc.vector.memset(dst[:, 0:1], 0.0)
                                deferred.append((p, w16_d, dst, q0))
                                continue
                            wp = proj_unit_dma(p, w_d)
                            proj_unit_mm(wp, dst, q0, xa, on_act=True)
            proj_ctx.close()

            # ---- phase C: attention + output projection ----
            # Head-phases: one head at a time, 16 QK+exp slots each. PV
            # half-passes of the previous head, out-proj chunks of the
            # previous q block, and the deferred half-1 Q/K projections are
            # interleaved into the PE gaps left by the ACT-paced exp stream.
            # E tiles double-buffer by head parity.
            if variant in ("proj_only", "qk_only", "v_only"):
                defer_ctx.close()
                continue
            # deadline order: K before Q, pair-major; half0 before half1 of
            # the same pair (kt needs both halves by phase 2p, qt half1 only
            # by qt1). deferred was appended (half, pair, w)-major; re-sort:
            if deferred:
                def _dl(u):
                    p_, w_d_, dst_, q0_ = u
                    is_q = w_d_ is wq16_d
                    h1 = bool(q0_)
                    # kt (both halves) and qt-half0 due at phase 2p of qt0;
                    # qt-half1 due at phase 2p of qt1
                    return (2 * p_ + (16 if (is_q and h1) else 0), is_q, h1)
                deferred.sort(key=_dl)
            subq_all = [(i, nt) for i in range(len(deferred)) for nt in range(2)]
            defer_wp = {}
            xr_sb = {}
            sub_emitted = 0

            def defer_xr_load():
                """One [128, 8x1024] f16 x-slab per half, loaded once."""
                for q0_ in (0, 1024):
                    xr = xr_pool.tile([128, 8192], dt.float16, name="xr", tag=f"xr{q0_}")
                    nc.sync.dma_start(
                        out=xr[:].rearrange("p (c m) -> p c m", m=1024),
                        in_=xT16_d[:, q0_ : q0_ + 1024].rearrange(
                            "(c p) m -> p c m", p=128
                        ),
                    )
                    xr_sb[q0_] = xr

            def defer_emit():
                nonlocal sub_emitted
                i_, nt_ = subq_all[sub_emitted]
                p_, w16_d_, dst_, q0_ = deferred[i_]
                if i_ not in defer_wp:
                    defer_wp[i_] = proj_unit_dma(p_, w16_d_, to_f16=True)
                wp = defer_wp[i_]
                xr = xr_sb[q0_]
                ps = psF.tile([128, 512], dt.float32, name="psA", tag="F")
                for c in range(8):
                    nc.tensor.matmul(
                        ps[:],
                        lhsT=wp[:, c * 128 : (c + 1) * 128],
                        rhs=xr[:, c * 1024 + nt_ * 512 : c * 1024 + (nt_ + 1) * 512],
                        start=(c == 0),
                        stop=(c == 7),
                    )
                nc.vector.tensor_copy(
                    dst_[:, q0_ + nt_ * 512 : q0_ + (nt_ + 1) * 512], ps[:]
                )
                if nt_ == 1:
                    defer_wp.pop(i_)
                    if i_ + 1 < len(deferred):
                        defer_wp[i_ + 1] = proj_unit_dma(
                            deferred[i_ + 1][0], deferred[i_ + 1][1], to_f16=True
                        )
                sub_emitted += 1

            with tc.tile_pool(name=f"mask{rep}", bufs=16) as mask_pool, tc.tile_pool(
                name=f"et{rep}", bufs=1
            ) as et_pool, tc.tile_pool(name=f"ep{rep}", bufs=2) as ep_pool, tc.tile_pool(
                name=f"osb{rep}", bufs=1
            ) as osb_pool:
                pv_po = {}

                def emit_pv_halfpass(state, hp):
                    """Two contiguous 16-matmul accumulation chains; on the
                    closing half also normalize (+ transpose on odd heads)."""
                    sqt, hh, eset = state
                    p, ho = hh // 2, hh % 2
                    ps4, half = hp // 2, hp % 2
                    if half == 0:
                        pv_po["po"] = psPV.tile([128, 512], dt.float32, name="po", tag="PV")
                    po = pv_po["po"]
                    for sub in (0, 1) if half == 0 else (2, 3):
                        qc = ps4 * 4 + sub
                        for k in range(NKT):
                            nc.tensor.matmul(
                                po[:, sub * 128 : sub * 128 + 65],
                                lhsT=eset[k][:, qc * 128 : (qc + 1) * 128],
                                rhs=vv_sb[k][:, hh * VW : hh * VW + 65],
                                start=(k == 0),
                                stop=(k == NKT - 1),
                            )
                    if half == 0:
                        return
                    if hh % 2 == 0 and hp == 1:
                        # fresh osb set per head pair (bufs=2 ping-pong) so the
                        # deferred transposes of the previous pair can still
                        # read theirs
                        pv_po["osb"] = [
                            osb_pool.tile([128, 128], dt.float16, name="osb", tag=f"o{i}")
                            for i in range(QW // 128)
                        ]
                    osb_t = pv_po["osb"]
                    rec = ep_pool.tile([128, 4], dt.float32, name="rec", tag="rec")
                    nc.vector.reciprocal(
                        rec[:], po[:].rearrange("p (s c) -> p s c", c=128)[:, :, 64]
                    )
                    for sub in range(4):
                        qc = ps4 * 4 + sub
                        nc.vector.tensor_scalar_mul(
                            osb_t[qc][:, ho * 64 : (ho + 1) * 64],
                            po[:, sub * 128 : sub * 128 + 64],
                            rec[:, sub : sub + 1],
                        )
                    if ho == 1 and hp == 3:
                        for qc in range(8):
                            tr_queue.append((p, sqt, qc, osb_t[qc]))
                        if hh == 7:
                            for qc in range(8):
                                op_queue.append((sqt, qc))

                def emit_transpose():
                    p_, sqt_, qc_, src = tr_queue.pop(0)
                    nc.sync.dma_start_transpose(
                        out=ot_sb[p_][:, sqt_ * QW + qc_ * 128 : sqt_ * QW + (qc_ + 1) * 128],
                        in_=src[:],
                    )

                def emit_outproj(sqt, qc):
                    q0 = sqt * QW + qc * 128
                    for ncol in range(2):
                        pf = psF.tile([128, 512], dt.float32, name="psF", tag="F")
                        for p in range(NPAIR):
                            nc.tensor.matmul(
                                pf[:],
                                lhsT=ot_sb[p][:, q0 : q0 + 128],
                                rhs=wo_sb[p][:, ncol * 512 : (ncol + 1) * 512],
                                start=(p == 0),
                                stop=(p == NPAIR - 1),
                            )
                        fsb = wpf_pool.tile([128, 512], dt.float16, name="fsb", tag="wpf")
                        nc.vector.tensor_copy(fsb[:], pf[:])
                        nc.sync.dma_start(
                            out=out_d[q0 : q0 + 128, ncol * 512 : (ncol + 1) * 512],
                            in_=fsb[:],
                        )

                prev = None
                op_queue = []
                tr_queue = []
                for qt in range(NQT):
                    mtiles = []
                    for k in range(NKT):
                        mt = mask_pool.tile([128, QW], dt.float16, name="mt", tag="mt")
                        nc.sync.dma_start(
                            out=mt[:], in_=mT_d[k * 128 : (k + 1) * 128, qt * QW : (qt + 1) * QW]
                        )
                        mtiles.append(mt)
                        if qt == 0 and k == 3 and deferred:
                            defer_xr_load()
                    for hh in range(8):
                        p, ho = hh // 2, hh % 2
                        b0 = ho * 64
                        eset = {}
                        for k in range(NKT):
                            pl = psL.tile([128, QW], dt.float32, name="psL", tag="L")
                            for hf in range(QW // 512):
                                nc.tensor.matmul(
                                    pl[:, hf * 512 : (hf + 1) * 512],
                                    lhsT=kt_sb[p][b0 : b0 + 64, k * 128 : (k + 1) * 128],
                                    rhs=qt_sb[p][
                                        b0 : b0 + 64,
                                        qt * QW + hf * 512 : qt * QW + (hf + 1) * 512,
                                    ],
                                    start=True,
                                    stop=True,
                                )
                            e = et_pool.tile(
                                [128, QW], dt.float16, name="et", tag=f"e{k}_{hh % 2}"
                            )
                            nc.scalar.activation(
                                e[:], pl[:], AF.Copy if variant == "no_exp" else AF.Exp
                            )
                            if variant != "no_mask":
                                nc.vector.tensor_mul(e[:], e[:], mtiles[k][:])
                            eset[k] = e
                            if k % 4 == 3 and prev is not None:
                                emit_pv_halfpass(prev, k // 4)
                            if k in (0, 2, 4, 8) and tr_queue:
                                emit_transpose()
                                emit_transpose()
                            if k in (6, 14) and op_queue:
                                emit_outproj(*op_queue.pop(0))
                            eligible = (
                                k in (7, 9, 11, 13, 15)
                                if (qt == 0 and hh == 0)
                                else k in (1, 2, 5, 6, 9, 10)
                            )
                            if sub_emitted < len(subq_all) and eligible:
                                defer_emit()
                        prev = (qt, hh, eset)
                # tail: last head's PV, then remaining out-proj chunks
                for hp in range(4):
                    emit_pv_halfpass(prev, hp)
                    while tr_queue:
                        emit_transpose()
                    if hp >= 1:
                        for _ in range(2):
                            if op_queue:
                                emit_outproj(*op_queue.pop(0))
                while op_queue:
                    emit_outproj(*op_queue.pop(0))
            defer_ctx.close()

    nc.compile()
    return nc


def _get_nc():
    if "nc" not in _CACHE:
        _CACHE["nc"] = _build()
    return _CACHE["nc"]


def kernel(x, mask, Wq, Wk, Wv, Wo, bo):
    x = np.asarray(x, dtype=np.float32)
    mask_f16 = np.asarray(mask).astype(np.float16)
    Wq = np.asarray(Wq, dtype=np.float32)
    Wk = np.asarray(Wk, dtype=np.float32)
    Wv = np.asarray(Wv, dtype=np.float32)
    Wo = np.asarray(Wo, dtype=np.float32)
    bo = np.asarray(bo, dtype=np.float32)

    scale = np.float32(DH**-0.5)
    nc = _get_nc()

    in_maps = []
    for c in range(8):
        b, g = c // 2, c % 2
        gs = slice(g * DG, (g + 1) * DG)
        in_maps.append(
            {
                "xT": np.ascontiguousarray(x[b].T),
                "wq": np.ascontiguousarray(Wq[:, gs]) * scale,
                "wk": np.ascontiguousarray(Wk[:, gs]),
                "wv": np.ascontiguousarray(Wv[:, gs]),
                "wo": np.ascontiguousarray(Wo[gs, :]).astype(np.float16),
                "maskT": np.ascontiguousarray(mask_f16[b].T),
                "wq16": (np.ascontiguousarray(Wq[:, gs]) * scale).astype(np.float16),
                "wk16": np.ascontiguousarray(Wk[:, gs]).astype(np.float16),
                "xT16": np.ascontiguousarray(x[b].T).astype(np.float16),
            }
        )

    res = run_bass_kernel_spmd(nc, in_maps, list(range(8))).results

    out = np.empty((B, Q, D), dtype=np.float32)
    for b in range(B):
        out[b] = res[2 * b]["out"].astype(np.float32) + res[2 * b + 1]["out"].astype(
            np.float32
        )
    out += bo
    return out

